# revision 25
# baseline (speedup 1.0000x reference)
"""AttentionalSplatting Trainium2 kernel (8 NeuronCores, SPMD).

Sharding: 8 cores = T(4) x HW-halves(2).  Core c handles t = c//2 and pixel
columns [ (c%2)*1152, (c%2+1)*1152 ).  Each core runs the full pipeline for
its (t, pixel-half): coord-proj + 2D RoPE -> Q/K/V proj -> qk-norm ->
scores(+spatial bias) -> softmax -> attend -> Wo -> W_out -> residual.
No cross-core communication is needed; outputs concatenate.

On-chip layout is feature-major ("transposed"): feature/head dims live on
SBUF partitions, pixels/tokens on the free dim.  Scores are computed as
S^T (m on partitions, q free) so the attend matmul consumes exp(S^T)
directly and softmax sums arrive free via a ones-column appended to V.

Attention loop: q is tiled in 3 blocks of 384, heads in two halves of 4.
Each sub-round handles one head-pair x one m-chunk: two identity matmuls
inject the (head-shared) spatial bias into a 2-bank PSUM tile, two
row-tiled (K=32) score matmuls accumulate on top concurrently, one scalar
Exp act (FD=768) produces bf16 E, and the attend matmuls (col-tiled pair)
are emitted with a 1-sub-round lag so the PE never stalls on the scalar
engine.  PSUM budget: score tiles 2 banks x2 bufs + attend accumulators
1 bank x4 = 8 banks.
"""

import math
import sys

import numpy as np

sys.path.insert(0, "/opt/trn_rl_repo")

import ml_dtypes  # noqa: E402

import concourse.bass as bass  # noqa: E402
import concourse.bacc as bacc  # noqa: E402
import concourse.tile as tile  # noqa: E402
from concourse import mybir  # noqa: E402
from concourse.bass_utils import run_bass_kernel_spmd  # noqa: E402

T, M, HW, DF, H = 4, 1024, 2304, 256, 8
DKH = DF // H  # 32
QH = HW // 2  # 1152 pixels per core
SCALE = 1.0 / math.sqrt(DKH)
D_HALF = DF // 2  # 128
D_QUART = DF // 4  # 64
THETA = (100.0 ** (-2.0 * np.arange(D_QUART, dtype=np.float32) / D_HALF)).astype(
    np.float32
)

F32 = mybir.dt.float32
BF16 = mybir.dt.bfloat16
AF = mybir.ActivationFunctionType
BF16NP = ml_dtypes.bfloat16

N_CORES = 8
QB = 384
Q_BLOCKS = [(0, QB), (QB, QB), (2 * QB, QB)]
K_CHUNKS = [(0, 512), (512, 512)]


def _bf(x):
    return np.ascontiguousarray(np.asarray(x, np.float32)).astype(BF16NP)


def _f32(x):
    return np.ascontiguousarray(np.asarray(x, np.float32))


def _host_constants(Wq, Wk, Wv, Wo, W_out_w, W_out_b, W_coord_w, W_coord_b):
    """Shared (core-independent) device constants, host-precomputed."""
    # pair-swapped coord weights for RoPE (swap even/odd output columns)
    perm = np.arange(DF)
    perm = perm.reshape(-1, 2)[:, ::-1].reshape(-1)
    # coord proj with bias folded in via the ones-row of pos_ang
    wc3 = np.concatenate([W_coord_w, W_coord_b[None, :]], axis=0)  # (3, DF)
    wcsw3 = np.concatenate([W_coord_w[:, perm], W_coord_b[perm][None, :]], axis=0)
    # theta lhsT: row0 = signed duplicated theta [-t0,+t0,...], row1 = pi/2
    thp = np.empty((2, D_HALF), np.float32)
    thp[0, 0::2] = -THETA
    thp[0, 1::2] = THETA
    thp[1, :] = math.pi / 2.0
    # block-ones for per-head sum of squares: dtile k maps its 128 feature
    # rows onto head columns 4k..4k+3
    bones = np.zeros((2, 128, 8), np.float32)
    for k in range(2):
        for d in range(128):
            bones[k, d, 4 * k + d // 32] = 1.0
    # expand per-head scalars (8, q) back to the 128 feature rows of dtile k
    exp8 = np.zeros((2, 8, 128), np.float32)
    for k in range(2):
        for d in range(128):
            exp8[k, 4 * k + d // 32, d] = 1.0
    # expand per-head inv-sum (8, q) to paired attend-output rows:
    # pair j holds head 2j at rows 1..33 and head 2j+1 at rows 65..97
    expP = np.zeros((4, 8, 128), np.float32)
    for j in range(4):
        expP[j, 2 * j, 1:33] = 1.0
        expP[j, 2 * j + 1, 65:97] = 1.0
    # Wo rearranged to the paired attend-output row layout (sumexp rows = 0)
    wo_aug = np.zeros((4, 128, DF), np.float32)
    for j in range(4):
        wo_aug[j, 1:33, :] = Wo[(2 * j) * 32 : (2 * j + 1) * 32, :]
        wo_aug[j, 65:97, :] = Wo[(2 * j + 1) * 32 : (2 * j + 2) * 32, :]
    def hi_lo(x):
        h = _bf(x)
        l = _bf(np.asarray(x, np.float32) - np.asarray(h, np.float32))
        return h, l

    thph, thpl = hi_lo(thp)
    return {
        "wq": _bf(Wq),
        "wk": _bf(Wk),
        "wv": _bf(Wv),
        "wo_aug": _bf(wo_aug),
        "wout": _bf(W_out_w),
        "woutb": _bf(W_out_b).reshape(1, DF),
        "wc3": _bf(wc3),
        "wcsw3": _bf(wcsw3),
        "thph": thph,
        "thpl": thpl,
        "bones": _bf(bones),
        "exp8": _bf(exp8),
        "expP": _bf(expP),
        "ident": _bf(np.eye(128, dtype=np.float32)),
    }


_NC_CACHE = None


def _build_nc():
    nc = bacc.Bacc(
        "TRN2",
        target_bir_lowering=False,
        debug=False,
        enable_asserts=True,
        num_devices=N_CORES,
    )
    d = {}
    inp = lambda name, shape, dt: d.__setitem__(
        name, nc.declare_dram_parameter(name, list(shape), dt, isOutput=False)
    )
    inp("tokT", (DF, M), BF16)
    inp("posAh", (3, QH), BF16)
    inp("posAl", (3, QH), BF16)
    inp("biasT", (M, QH), BF16)
    inp("fmapT", (DF, QH), F32)
    inp("wq", (DF, DF), BF16)
    inp("wk", (DF, DF), BF16)
    inp("wv", (DF, DF), BF16)
    inp("wo_aug", (4, 128, DF), BF16)
    inp("wout", (DF, DF), BF16)
    inp("woutb", (1, DF), BF16)
    inp("wc3", (3, DF), BF16)
    inp("wcsw3", (3, DF), BF16)
    inp("thph", (2, D_HALF), BF16)
    inp("thpl", (2, D_HALF), BF16)
    inp("bones", (2, 128, 8), BF16)
    inp("exp8", (2, 8, 128), BF16)
    inp("expP", (4, 8, 128), BF16)
    inp("ident", (128, 128), BF16)
    out = nc.declare_dram_parameter("out", [DF, QH], F32, isOutput=True)

    import os as _os

    with tile.TileContext(
        nc, trace_sim=bool(_os.environ.get("KERNEL_TRACE_SIM"))
    ) as tc:
        _body(nc, tc, d, out)
    nc.compile()
    return nc


def _body(nc, tc, d, out_dram):
    mm = nc.tensor.matmul
    act = nc.scalar.activation
    dma = nc.sync.dma_start

    with (
        tc.tile_pool(name="const", bufs=1) as cpool,
        tc.tile_pool(name="work", bufs=1) as wpool,
        tc.tile_pool(name="persist", bufs=1) as ppool,
        tc.tile_pool(name="epool", bufs=6) as epool,
        tc.tile_pool(name="psc", bufs=2, space=bass.MemorySpace.PSUM) as psc,
        tc.tile_pool(name="pso", bufs=2, space=bass.MemorySpace.PSUM) as pso,
    ):
        # ---- load constants / inputs to SBUF ----
        # 256-row tensors fold to (128, 2, ...): [:, kt, ...] = rows kt*128..
        def load(name, shape, dt, rearrange=None, **kw):
            t = cpool.tile(list(shape), dt, tag=name)
            src = d[name][:]
            if rearrange is not None:
                src = src.rearrange(rearrange, **kw)
            dma(t[:], src)
            return t

        fold = "(k p) d -> p k d"
        # critical-path loads first (pre-phase + attention start)
        # position rows in hi+lo bf16 split (fp32 matmuls are 4x slower)
        posAh = load("posAh", (3, QH), BF16)
        posAl = load("posAl", (3, QH), BF16)
        # rows (y, ones) re-based to partition 0 for the axis-1 angle mms
        posBh = cpool.tile([2, QH], BF16, tag="posBh")
        dma(posBh[:], d["posAh"][1:3, :])
        posBl = cpool.tile([2, QH], BF16, tag="posBl")
        dma(posBl[:], d["posAl"][1:3, :])
        # rows (x, ones) adjacent for the axis-0 cos mm
        posCh = cpool.tile([2, QH], BF16, tag="posCh")
        dma(posCh[0:1, :], d["posAh"][0:1, :])
        dma(posCh[1:2, :], d["posAh"][2:3, :])
        posCl = cpool.tile([2, QH], BF16, tag="posCl")
        dma(posCl[0:1, :], d["posAl"][0:1, :])
        dma(posCl[1:2, :], d["posAl"][2:3, :])
        thph = load("thph", (2, D_HALF), BF16)
        thpl = load("thpl", (2, D_HALF), BF16)
        wc3 = load("wc3", (3, DF), BF16)
        wcsw3 = load("wcsw3", (3, DF), BF16)
        ident = load("ident", (128, 128), BF16)
        bones = load("bones", (128, 2, 8), BF16, "k p h -> p k h")
        exp8 = load("exp8", (8, 2, 128), BF16, "k h d -> h k d")
        tokT = load("tokT", (128, 2, M), BF16, fold, p=128)
        wk = load("wk", (128, 2, DF), BF16, fold, p=128)
        wv = load("wv", (128, 2, DF), BF16, fold, p=128)
        wq = load("wq", (128, 2, DF), BF16, fold, p=128)

        bias_sb = []
        for mc in range(8):
            bt = ppool.tile([128, QH], BF16, tag=f"bias{mc}")
            dma(bt[:], d["biasT"][mc * 128 : (mc + 1) * 128, :])
            bias_sb.append(bt)

        # tail-phase loads (emitted late in the DMA queue on purpose)
        wo_aug = load("wo_aug", (128, 4, DF), BF16, "j p d -> p j d")
        wout = load("wout", (128, 2, DF), BF16, fold, p=128)
        woutb = load("woutb", (1, 2, 128), BF16, "o (k p) -> o k p", k=2)
        expP = load("expP", (8, 4, 128), BF16, "j s e -> s j e")
        fmapT = load("fmapT", (128, 2, QH), F32, fold, p=128)

        ones_q = cpool.tile([1, QB], BF16, tag="ones_q")
        nc.vector.memset(ones_q[:], 1.0)

        # ---- phase A: angle tables, all Sin acts first (one table set) ----
        # sc-tile layout [128, 2, 512]: [:,0,:]=sin(angles), [:,1,:]=cos
        cs = []  # cs[axis] = bf16 [128, 2, QH] (sin, cos)
        for axis in range(2):
            c = ppool.tile([128, 2, QH], BF16, tag=f"cs{axis}")
            cs.append(c)
        for qo, qb in Q_BLOCKS:
            for axis in range(2):
                ps = psc.tile([128, 2, 512], F32, tag="sc")
                sh = posAh if axis == 0 else posBh
                sl = posAl if axis == 0 else posBl
                mm(ps[:, 0, 0:qb], thph[0:1, :], sh[0:1, qo : qo + qb],
                   start=True, stop=False)
                mm(ps[:, 0, 0:qb], thph[0:1, :], sl[0:1, qo : qo + qb],
                   start=False, stop=False)
                mm(ps[:, 0, 0:qb], thpl[0:1, :], sh[0:1, qo : qo + qb],
                   start=False, stop=True)
                # cos half: theta*pos + (pi/2)*1 via 2-row contraction
                rh = (posCh if axis == 0 else posBh)[0:2, qo : qo + qb]
                rl = (posCl if axis == 0 else posBl)[0:2, qo : qo + qb]
                mm(ps[:, 1, 0:qb], thph[:, :], rh, start=True, stop=False)
                mm(ps[:, 1, 0:qb], thph[:, :], rl, start=False, stop=False)
                mm(ps[:, 1, 0:qb], thpl[:, :], rh, start=False, stop=True)
                act(cs[axis][:, :, qo : qo + qb], ps[:, :, 0:qb], AF.Sin)

        knT = ppool.tile([128, 2, M], BF16, tag="knT")
        qnT = ppool.tile([128, 2, QH], BF16, tag="qnT")
        lnscale = cpool.tile([128, 1], F32, tag="lnscale")
        nc.vector.memset(lnscale[:], math.log(SCALE))
        zero_c = cpool.tile([128, 1], F32, tag="zero_c")
        nc.vector.memset(zero_c[:], 0.0)
        # scalar-written fences force Sin -> Ln -> Exp engine order so the
        # lower_act pass emits one table load per set (no thrashing).
        # Each fence is Copy(0*x + bias) of the prior phase's last output:
        # zero/const valued, but data-dependent so the scheduler keeps order.
        one_c = cpool.tile([128, 1], F32, tag="one_c")
        nc.vector.memset(one_c[:], 1.0)
        scale_c = cpool.tile([128, 1], F32, tag="scale_c")
        nc.vector.memset(scale_c[:], SCALE)
        fence_ln = cpool.tile([128, 1], F32, tag="fence_ln")
        act(fence_ln[:], cs[1][:, 1, QH - 1 : QH], AF.Sin, scale=0.0,
            bias=zero_c[:])
        fence_exp0 = cpool.tile([128, 1], F32, tag="fence_exp0")
        fence_expS = cpool.tile([128, 1], F32, tag="fence_expS")

        # qk-norm runs in two passes so the scalar engine sees all Ln acts
        # then all Exp acts (one table load each).
        segs = []  # (tb, lnt, out_tile, off, n, ln_bias)

        def norm_pass1(ps, n, idx):
            tb = wpool.tile([128, 2, 512], BF16, tag=f"tb{idx}")
            nc.vector.tensor_copy(tb[:, :, 0:n], ps[:, :, 0:n])
            sq = wpool.tile([128, 2, 512], BF16, tag="sq", bufs=2)
            nc.vector.tensor_mul(sq[:, :, 0:n], tb[:, :, 0:n], tb[:, :, 0:n])
            sq_ps = pso.tile([128, 512], F32, tag="op0")
            for dt_i in range(2):
                mm(
                    sq_ps[0:8, 0:n],
                    bones[:, dt_i, :],
                    sq[:, dt_i, 0:n],
                    start=(dt_i == 0),
                    stop=(dt_i == 1),
                )
            lnt = wpool.tile([8, 512], F32, tag=f"lnt{idx}")
            act(lnt[:, 0:n], sq_ps[0:8, 0:n], AF.Ln, bias=fence_ln[0:8, :])
            return tb, lnt

        # ---- K projection + norm pass 1 ----
        for ci, (ko, kb) in enumerate(K_CHUNKS):
            ps = psc.tile([128, 2, 512], F32, tag="sc")
            for dt_i in range(2):
                for kt in range(2):
                    mm(
                        ps[:, dt_i, 0:kb],
                        wk[:, kt, dt_i * 128 : (dt_i + 1) * 128],
                        tokT[:, kt, ko : ko + kb],
                        start=(kt == 0),
                        stop=(kt == 1),
                    )
            tb, lnt = norm_pass1(ps, kb, f"k{ci}")
            segs.append((tb, lnt, knT, ko, kb, zero_c))

        # ---- phase B: coord proj (bias folded) + RoPE (DVE only) ----
        roped = wpool.tile([128, 2, QH], BF16, tag="roped")
        for qo, qb in Q_BLOCKS:
            pq = psc.tile([128, 2, 512], F32, tag="sc")
            pqs = psc.tile([128, 2, 512], F32, tag="sc")
            for dt_i in range(2):
                for w_t, ps_t in ((wc3, pq), (wcsw3, pqs)):
                    mm(
                        ps_t[:, dt_i, 0:qb],
                        w_t[:, dt_i * 128 : (dt_i + 1) * 128],
                        posAh[:, qo : qo + qb],
                        start=True,
                        stop=False,
                    )
                    mm(
                        ps_t[:, dt_i, 0:qb],
                        w_t[:, dt_i * 128 : (dt_i + 1) * 128],
                        posAl[:, qo : qo + qb],
                        start=False,
                        stop=True,
                    )
            qin = wpool.tile([128, 2, 512], BF16, tag="qin", bufs=2)
            qins = wpool.tile([128, 2, 512], BF16, tag="qins", bufs=2)
            nc.vector.tensor_copy(qin[:, :, 0:qb], pq[:, :, 0:qb])
            nc.vector.tensor_copy(qins[:, :, 0:qb], pqs[:, :, 0:qb])
            t1 = wpool.tile([128, 2, 512], BF16, tag="t1", bufs=2)
            t2 = wpool.tile([128, 2, 512], BF16, tag="t2", bufs=2)
            for dt_i in range(2):
                # dt0 <- x-axis tables, dt1 <- y-axis
                nc.vector.tensor_mul(
                    t1[:, dt_i, 0:qb],
                    qin[:, dt_i, 0:qb],
                    cs[dt_i][:, 1, qo : qo + qb],
                )
                nc.vector.tensor_mul(
                    t2[:, dt_i, 0:qb],
                    qins[:, dt_i, 0:qb],
                    cs[dt_i][:, 0, qo : qo + qb],
                )
            nc.vector.tensor_add(
                roped[:, :, qo : qo + qb], t1[:, :, 0:qb], t2[:, :, 0:qb]
            )

        # ---- Q projection + norm pass 1 ----
        for qi, (qo, qb) in enumerate(Q_BLOCKS):
            ps = psc.tile([128, 2, 512], F32, tag="sc")
            for dt_i in range(2):
                for kt in range(2):
                    mm(
                        ps[:, dt_i, 0:qb],
                        wq[:, kt, dt_i * 128 : (dt_i + 1) * 128],
                        roped[:, kt, qo : qo + qb],
                        start=(kt == 0),
                        stop=(kt == 1),
                    )
            tb, lnt = norm_pass1(ps, qb, f"q{qi}")
            segs.append((tb, lnt, qnT, qo, qb, lnscale))

        # ---- norm pass 2: all Exp acts, then expand + scale ----
        last_lnt = segs[-1][1]
        # Ln(0*x + 1) == 0 and Ln(0*x + SCALE) == ln(SCALE); both stay in
        # the natural_log table set while depending on the last Ln output.
        act(fence_exp0[0:8, :], last_lnt[0:8, 0:1], AF.Ln, scale=0.0,
            bias=one_c[0:8, :])
        act(fence_expS[0:8, :], last_lnt[0:8, 0:1], AF.Ln, scale=0.0,
            bias=scale_c[0:8, :])
        for si, (tb, lnt, out_t, off, n, ln_bias) in enumerate(segs):
            ln_bias = fence_expS if ln_bias is lnscale else fence_exp0
            invn = wpool.tile([8, 512], BF16, tag="invn", bufs=3)
            act(
                invn[:, 0:n], lnt[:, 0:n], AF.Exp, scale=-0.5, bias=ln_bias[0:8, :]
            )
            for dt_i in range(2):
                psx = pso.tile([128, 512], F32, tag="op1")
                mm(psx[:, 0:n], exp8[:, dt_i, :], invn[:, 0:n])
                nc.vector.tensor_mul(
                    out_t[:, dt_i, off : off + n], tb[:, dt_i, 0:n], psx[:, 0:n]
                )

        # ---- V (token-major) with ones column:  vsb[mc] = (128, 8, 33) ----
        vsb = []
        for mc in range(8):
            ps = pso.tile([128, 512], F32, tag="op1")
            for kt in range(2):
                mm(
                    ps[:, 0:256],
                    tokT[:, kt, mc * 128 : (mc + 1) * 128],
                    wv[:, kt, :],
                    start=(kt == 0),
                    stop=(kt == 1),
                )
            vt = ppool.tile([128, 8, 33], BF16, tag=f"v{mc}")
            nc.vector.memset(vt[:, :, 0:1], 1.0)
            nc.vector.tensor_copy(
                vt[:, :, 1:33], ps[:, 0:256].rearrange("p (h e) -> p h e", h=8)
            )
            vsb.append(vt)

        # ---- main attention loop ----
        osb = []  # per pair (128, QH) bf16, rows 0/64 = sumexp
        for j in range(4):
            t = ppool.tile([128, QH], BF16, tag=f"osb{j}")
            osb.append(t)

        for qo, qb in Q_BLOCKS:
            for half in range(2):
                dt_i = half
                o_ps = {}
                for jj in range(2):
                    j = 2 * half + jj
                    o_ps[j] = pso.tile(
                        [128, 512], F32, tag=f"op{jj}", name=f"ops{j}"
                    )
                pend = []

                def emit_attend(item):
                    j, mc, e_t = item
                    h0, h1 = 2 * j, 2 * j + 1
                    mm(
                        o_ps[j][0:33, 0:qb],
                        vsb[mc][:, h0, :],
                        e_t[:, 0, 0:qb],
                        start=(mc == 0),
                        stop=(mc == 7),
                        tile_position=(0, 0),
                    )
                    mm(
                        o_ps[j][64:97, 0:qb],
                        vsb[mc][:, h1, :],
                        e_t[:, 1, 0:qb],
                        start=(mc == 0),
                        stop=(mc == 7),
                        tile_position=(0, 64),
                    )

                for mc in range(8):
                    # one cluster: both head-pairs of this half x one m-chunk
                    Xs = []
                    for jj in range(2):
                        X = psc.tile([128, 2, 512], F32, tag="sc", name=f"x{jj}")
                        Xs.append(X)
                    # batched bias injection: 4 mms sharing the ident weights
                    for jj in range(2):
                        for hh in range(2):
                            mm(
                                Xs[jj][:, hh, 0:qb],
                                ident[:],
                                bias_sb[mc][:, qo : qo + qb],
                                start=True,
                                stop=False,
                            )
                    # batched scores: 4 row-tiled mms run concurrently
                    for jj in range(2):
                        j = 2 * half + jj
                        for hh in range(2):
                            h = 2 * j + hh
                            hp = (h % 4) * 32
                            mm(
                                Xs[jj][:, hh, 0:qb],
                                knT[hp : hp + 32, dt_i, mc * 128 : (mc + 1) * 128],
                                qnT[hp : hp + 32, dt_i, qo : qo + qb],
                                start=False,
                                stop=True,
                                tile_position=(hp, 0),
                            )
                    for jj in range(2):
                        j = 2 * half + jj
                        e_t = epool.tile([128, 2, 512], BF16, tag="E")
                        act(e_t[:, :, 0:qb], Xs[jj][:, :, 0:qb], AF.Exp)
                        pend.append((j, mc, e_t))
                    if len(pend) > 2:
                        emit_attend(pend.pop(0))
                        emit_attend(pend.pop(0))
                while pend:
                    emit_attend(pend.pop(0))
                for jj in range(2):
                    j = 2 * half + jj
                    nc.vector.tensor_copy(
                        osb[j][:, qo : qo + qb], o_ps[j][:, 0:qb]
                    )

        # ---- softmax denominators: gather row 0 of each head, invert ----
        sumE = wpool.tile([8, QH], BF16, tag="sumE")
        for h in range(8):
            j, r = h // 2, 64 * (h % 2)
            dma(sumE[h : h + 1, :], osb[h // 2][r : r + 1, :])
        sumEf = wpool.tile([8, QH], F32, tag="sumEf")
        nc.vector.tensor_copy(sumEf[:], sumE[:])
        invS = wpool.tile([8, QH], F32, tag="invS")
        nc.vector.reciprocal(invS[:], sumEf[:])
        invSb = wpool.tile([8, QH], BF16, tag="invSb")
        nc.vector.tensor_copy(invSb[:], invS[:])

        for qo, qb in Q_BLOCKS:
            for j in range(4):
                ps = pso.tile([128, 512], F32, tag=f"op{j % 2}")
                mm(ps[:, 0:qb], expP[:, j, :], invSb[:, qo : qo + qb])
                nc.vector.tensor_mul(
                    osb[j][:, qo : qo + qb], osb[j][:, qo : qo + qb], ps[:, 0:qb]
                )

        # ---- output projections + residual (all biases via matmul) ----
        for qo, qb in Q_BLOCKS:
            ps = psc.tile([128, 2, 512], F32, tag="sc")
            for dt_i in range(2):
                for j in range(4):
                    mm(
                        ps[:, dt_i, 0:qb],
                        wo_aug[:, j, dt_i * 128 : (dt_i + 1) * 128],
                        osb[j][:, qo : qo + qb],
                        start=(j == 0),
                        stop=(j == 3),
                    )
            o1b = wpool.tile([128, 2, 512], BF16, tag="o1b", bufs=2)
            nc.vector.tensor_copy(o1b[:, :, 0:qb], ps[:, :, 0:qb])
            ps2 = psc.tile([128, 2, 512], F32, tag="sc")
            for dt_i in range(2):
                for kt in range(2):
                    mm(
                        ps2[:, dt_i, 0:qb],
                        wout[:, kt, dt_i * 128 : (dt_i + 1) * 128],
                        o1b[:, kt, 0:qb],
                        start=(kt == 0),
                        stop=False,
                    )
                mm(
                    ps2[:, dt_i, 0:qb],
                    woutb[:, dt_i, :],
                    ones_q[:, 0:qb],
                    start=False,
                    stop=True,
                )
            res = wpool.tile([128, 2, 512], F32, tag="res", bufs=2)
            nc.vector.tensor_add(
                res[:, :, 0:qb], ps2[:, :, 0:qb], fmapT[:, :, qo : qo + qb]
            )
            for dt_i in range(2):
                dma(
                    out_dram[dt_i * 128 : (dt_i + 1) * 128, qo : qo + qb],
                    res[:, dt_i, 0:qb],
                )


def build_in_maps(inputs):
    consts = _host_constants(
        np.asarray(inputs["Wq"], np.float32),
        np.asarray(inputs["Wk"], np.float32),
        np.asarray(inputs["Wv"], np.float32),
        np.asarray(inputs["Wo"], np.float32),
        np.asarray(inputs["W_out_w"], np.float32),
        np.asarray(inputs["W_out_b"], np.float32),
        np.asarray(inputs["W_coord_w"], np.float32),
        np.asarray(inputs["W_coord_b"], np.float32),
    )
    track_tokens = np.asarray(inputs["track_tokens"], np.float32)
    feature_map = np.asarray(inputs["feature_map"], np.float32)
    feature_positions = np.asarray(inputs["feature_positions"], np.float32)
    spatial_bias = np.asarray(inputs["spatial_bias"], np.float32)

    in_maps = []
    for c in range(N_CORES):
        t, half = c // 2, c % 2
        qsl = slice(half * QH, (half + 1) * QH)
        m = dict(consts)
        m["tokT"] = _bf(track_tokens[t].T)
        pos = feature_positions[t, qsl].T  # (2, QH)
        posA = np.concatenate([pos, np.ones((1, QH), np.float32)], axis=0)
        ph = _bf(posA)
        m["posAh"] = ph
        m["posAl"] = _bf(posA - np.asarray(ph, np.float32))
        m["biasT"] = _bf(spatial_bias[t][:, qsl])
        m["fmapT"] = _f32(feature_map[t, qsl].T)
        in_maps.append(m)
    return in_maps


def kernel(
    track_tokens,
    feature_map,
    feature_positions,
    spatial_bias,
    Wq,
    Wk,
    Wv,
    Wo,
    W_out_w,
    W_out_b,
    W_coord_w,
    W_coord_b,
):
    global _NC_CACHE
    in_maps = build_in_maps(
        dict(
            track_tokens=track_tokens,
            feature_map=feature_map,
            feature_positions=feature_positions,
            spatial_bias=spatial_bias,
            Wq=Wq,
            Wk=Wk,
            Wv=Wv,
            Wo=Wo,
            W_out_w=W_out_w,
            W_out_b=W_out_b,
            W_coord_w=W_coord_w,
            W_coord_b=W_coord_b,
        )
    )

    if _NC_CACHE is None:
        _NC_CACHE = _build_nc()
    res = run_bass_kernel_spmd(_NC_CACHE, in_maps, core_ids=list(range(N_CORES)))

    out = np.empty((T, HW, DF), np.float32)
    for c in range(N_CORES):
        t, half = c // 2, c % 2
        qsl = slice(half * QH, (half + 1) * QH)
        out[t, qsl, :] = res.results[c]["out"].T
    return out


# revision 29
# speedup vs baseline: 1.0678x; 1.0678x over previous
"""AttentionalSplatting Trainium2 kernel (8 NeuronCores, SPMD).

Sharding: 8 cores = T(4) x HW-halves(2).  Core c handles t = c//2 and pixel
columns [ (c%2)*1152, (c%2+1)*1152 ).  Each core runs the full pipeline for
its (t, pixel-half): coord-proj + 2D RoPE -> Q/K/V proj -> qk-norm ->
scores(+spatial bias) -> softmax -> attend -> Wo -> W_out -> residual.
No cross-core communication is needed; outputs concatenate.

On-chip layout is feature-major ("transposed"): feature/head dims live on
SBUF partitions, pixels/tokens on the free dim.  Scores are computed as
S^T (m on partitions, q free) so the attend matmul consumes exp(S^T)
directly and softmax sums arrive free via a ones-column appended to V.

Attention loop: q is tiled in 3 blocks of 384, heads in two halves of 4.
Each sub-round handles one head-pair x one m-chunk: two identity matmuls
inject the (head-shared) spatial bias into a 2-bank PSUM tile, two
row-tiled (K=32) score matmuls accumulate on top concurrently, one scalar
Exp act (FD=768) produces bf16 E, and the attend matmuls (col-tiled pair)
are emitted with a 1-sub-round lag so the PE never stalls on the scalar
engine.  PSUM budget: score tiles 2 banks x2 bufs + attend accumulators
1 bank x4 = 8 banks.
"""

import math
import sys

import numpy as np

sys.path.insert(0, "/opt/trn_rl_repo")

import ml_dtypes  # noqa: E402

import concourse.bass as bass  # noqa: E402
import concourse.bacc as bacc  # noqa: E402
import concourse.tile as tile  # noqa: E402
from concourse import mybir  # noqa: E402
from concourse.bass_utils import run_bass_kernel_spmd  # noqa: E402

T, M, HW, DF, H = 4, 1024, 2304, 256, 8
DKH = DF // H  # 32
QH = HW // 2  # 1152 pixels per core
SCALE = 1.0 / math.sqrt(DKH)
D_HALF = DF // 2  # 128
D_QUART = DF // 4  # 64
THETA = (100.0 ** (-2.0 * np.arange(D_QUART, dtype=np.float32) / D_HALF)).astype(
    np.float32
)

F32 = mybir.dt.float32
BF16 = mybir.dt.bfloat16
AF = mybir.ActivationFunctionType
BF16NP = ml_dtypes.bfloat16

N_CORES = 8
Q_BLOCKS = [(0, 512), (512, 512), (1024, 128)]
K_CHUNKS = [(0, 512), (512, 512)]


def _bf(x):
    return np.ascontiguousarray(np.asarray(x, np.float32)).astype(BF16NP)


def _f32(x):
    return np.ascontiguousarray(np.asarray(x, np.float32))


def _host_constants(Wq, Wk, Wv, Wo, W_out_w, W_out_b, W_coord_w, W_coord_b):
    """Shared (core-independent) device constants, host-precomputed."""
    # pair-swapped coord weights for RoPE (swap even/odd output columns)
    perm = np.arange(DF)
    perm = perm.reshape(-1, 2)[:, ::-1].reshape(-1)
    # coord proj with bias folded in via the ones-row of pos_ang
    wc3 = np.concatenate([W_coord_w, W_coord_b[None, :]], axis=0)  # (3, DF)
    wcsw3 = np.concatenate([W_coord_w[:, perm], W_coord_b[perm][None, :]], axis=0)
    # theta lhsT: row0 = signed duplicated theta [-t0,+t0,...], row1 = pi/2
    thp = np.empty((2, D_HALF), np.float32)
    thp[0, 0::2] = -THETA
    thp[0, 1::2] = THETA
    thp[1, :] = math.pi / 2.0
    # block-ones for per-head sum of squares: dtile k maps its 128 feature
    # rows onto head columns 4k..4k+3
    bones = np.zeros((2, 128, 8), np.float32)
    for k in range(2):
        for d in range(128):
            bones[k, d, 4 * k + d // 32] = 1.0
    # expand per-head scalars (8, q) back to the 128 feature rows of dtile k
    exp8 = np.zeros((2, 8, 128), np.float32)
    for k in range(2):
        for d in range(128):
            exp8[k, 4 * k + d // 32, d] = 1.0
    # expand per-head inv-sum (8, q) to paired attend-output rows:
    # pair j holds head 2j at rows 1..33 and head 2j+1 at rows 65..97
    expP = np.zeros((4, 8, 128), np.float32)
    for j in range(4):
        expP[j, 2 * j, 1:33] = 1.0
        expP[j, 2 * j + 1, 65:97] = 1.0
    # Wo rearranged to the paired attend-output row layout (sumexp rows = 0)
    wo_aug = np.zeros((4, 128, DF), np.float32)
    for j in range(4):
        wo_aug[j, 1:33, :] = Wo[(2 * j) * 32 : (2 * j + 1) * 32, :]
        wo_aug[j, 65:97, :] = Wo[(2 * j + 1) * 32 : (2 * j + 2) * 32, :]
    def hi_lo(x):
        h = _bf(x)
        l = _bf(np.asarray(x, np.float32) - np.asarray(h, np.float32))
        return h, l

    thph, thpl = hi_lo(thp)
    return {
        "wq": _bf(Wq),
        "wk": _bf(Wk),
        "wv": _bf(Wv),
        "wo_aug": _bf(wo_aug),
        "wout": _bf(W_out_w),
        "woutb": _bf(W_out_b).reshape(1, DF),
        "wc3": _bf(wc3),
        "wcsw3": _bf(wcsw3),
        "thph": thph,
        "thpl": thpl,
        "bones": _bf(bones),
        "exp8": _bf(exp8),
        "expP": _bf(expP),
        "ident": _bf(np.eye(128, dtype=np.float32)),
    }


_NC_CACHE = None


def _build_nc():
    nc = bacc.Bacc(
        "TRN2",
        target_bir_lowering=False,
        debug=False,
        enable_asserts=True,
        num_devices=N_CORES,
    )
    d = {}
    inp = lambda name, shape, dt: d.__setitem__(
        name, nc.declare_dram_parameter(name, list(shape), dt, isOutput=False)
    )
    inp("tokT", (DF, M), BF16)
    inp("posAh", (3, QH), BF16)
    inp("posAl", (3, QH), BF16)
    inp("biasT", (M, QH), BF16)
    inp("fmapT", (DF, QH), F32)
    inp("wq", (DF, DF), BF16)
    inp("wk", (DF, DF), BF16)
    inp("wv", (DF, DF), BF16)
    inp("wo_aug", (4, 128, DF), BF16)
    inp("wout", (DF, DF), BF16)
    inp("woutb", (1, DF), BF16)
    inp("wc3", (3, DF), BF16)
    inp("wcsw3", (3, DF), BF16)
    inp("thph", (2, D_HALF), BF16)
    inp("thpl", (2, D_HALF), BF16)
    inp("bones", (2, 128, 8), BF16)
    inp("exp8", (2, 8, 128), BF16)
    inp("expP", (4, 8, 128), BF16)
    inp("ident", (128, 128), BF16)
    out = nc.declare_dram_parameter("out", [DF, QH], F32, isOutput=True)

    import os as _os

    with tile.TileContext(
        nc, trace_sim=bool(_os.environ.get("KERNEL_TRACE_SIM"))
    ) as tc:
        _body(nc, tc, d, out)
    nc.compile()
    return nc


def _body(nc, tc, d, out_dram):
    mm = nc.tensor.matmul
    act = nc.scalar.activation
    dma = nc.sync.dma_start

    with (
        tc.tile_pool(name="const", bufs=1) as cpool,
        tc.tile_pool(name="work", bufs=1) as wpool,
        tc.tile_pool(name="persist", bufs=1) as ppool,
        tc.tile_pool(name="epool", bufs=6) as epool,
        tc.tile_pool(name="psc", bufs=2, space=bass.MemorySpace.PSUM) as psc,
        tc.tile_pool(name="pso", bufs=2, space=bass.MemorySpace.PSUM) as pso,
    ):
        # ---- load constants / inputs to SBUF ----
        # 256-row tensors fold to (128, 2, ...): [:, kt, ...] = rows kt*128..
        def load(name, shape, dt, rearrange=None, **kw):
            t = cpool.tile(list(shape), dt, tag=name)
            src = d[name][:]
            if rearrange is not None:
                src = src.rearrange(rearrange, **kw)
            dma(t[:], src)
            return t

        fold = "(k p) d -> p k d"
        # critical-path loads first (pre-phase + attention start)
        # position rows in hi+lo bf16 split (fp32 matmuls are 4x slower)
        posAh = load("posAh", (3, QH), BF16)
        posAl = load("posAl", (3, QH), BF16)
        # rows (y, ones) re-based to partition 0 for the axis-1 angle mms
        posBh = cpool.tile([2, QH], BF16, tag="posBh")
        dma(posBh[:], d["posAh"][1:3, :])
        posBl = cpool.tile([2, QH], BF16, tag="posBl")
        dma(posBl[:], d["posAl"][1:3, :])
        # rows (x, ones) adjacent for the axis-0 cos mm
        posCh = cpool.tile([2, QH], BF16, tag="posCh")
        dma(posCh[0:1, :], d["posAh"][0:1, :])
        dma(posCh[1:2, :], d["posAh"][2:3, :])
        posCl = cpool.tile([2, QH], BF16, tag="posCl")
        dma(posCl[0:1, :], d["posAl"][0:1, :])
        dma(posCl[1:2, :], d["posAl"][2:3, :])
        thph = load("thph", (2, D_HALF), BF16)
        thpl = load("thpl", (2, D_HALF), BF16)
        wc3 = load("wc3", (3, DF), BF16)
        wcsw3 = load("wcsw3", (3, DF), BF16)
        ident = load("ident", (128, 128), BF16)
        bones = load("bones", (128, 2, 8), BF16, "k p h -> p k h")
        exp8 = load("exp8", (8, 2, 128), BF16, "k h d -> h k d")
        tokT = load("tokT", (128, 2, M), BF16, fold, p=128)
        wk = load("wk", (128, 2, DF), BF16, fold, p=128)
        wv = load("wv", (128, 2, DF), BF16, fold, p=128)
        wq = load("wq", (128, 2, DF), BF16, fold, p=128)

        bias_sb = []
        for mc in range(8):
            bt = ppool.tile([128, QH], BF16, tag=f"bias{mc}")
            dma(bt[:], d["biasT"][mc * 128 : (mc + 1) * 128, :])
            bias_sb.append(bt)

        # tail-phase loads (emitted late in the DMA queue on purpose)
        wo_aug = load("wo_aug", (128, 4, DF), BF16, "j p d -> p j d")
        wout = load("wout", (128, 2, DF), BF16, fold, p=128)
        woutb = load("woutb", (1, 2, 128), BF16, "o (k p) -> o k p", k=2)
        expP = load("expP", (8, 4, 128), BF16, "j s e -> s j e")
        fmapT = load("fmapT", (128, 2, QH), F32, fold, p=128)

        ones_q = cpool.tile([1, 512], BF16, tag="ones_q")
        nc.vector.memset(ones_q[:], 1.0)

        # ---- phase A: angle tables, all Sin acts first (one table set) ----
        # sc-tile layout [128, 2, 512]: [:,0,:]=sin(angles), [:,1,:]=cos
        cs = []  # cs[axis] = bf16 [128, 2, QH] (sin, cos)
        for axis in range(2):
            c = ppool.tile([128, 2, QH], BF16, tag=f"cs{axis}")
            cs.append(c)
        for qo, qb in Q_BLOCKS:
            for axis in range(2):
                ps = psc.tile([128, 2, 512], F32, tag="sc")
                sh = posAh if axis == 0 else posBh
                sl = posAl if axis == 0 else posBl
                mm(ps[:, 0, 0:qb], thph[0:1, :], sh[0:1, qo : qo + qb],
                   start=True, stop=False)
                mm(ps[:, 0, 0:qb], thph[0:1, :], sl[0:1, qo : qo + qb],
                   start=False, stop=False)
                mm(ps[:, 0, 0:qb], thpl[0:1, :], sh[0:1, qo : qo + qb],
                   start=False, stop=True)
                # cos half: theta*pos + (pi/2)*1 via 2-row contraction
                rh = (posCh if axis == 0 else posBh)[0:2, qo : qo + qb]
                rl = (posCl if axis == 0 else posBl)[0:2, qo : qo + qb]
                mm(ps[:, 1, 0:qb], thph[:, :], rh, start=True, stop=False)
                mm(ps[:, 1, 0:qb], thph[:, :], rl, start=False, stop=False)
                mm(ps[:, 1, 0:qb], thpl[:, :], rh, start=False, stop=True)
                act(cs[axis][:, :, qo : qo + qb], ps[:, :, 0:qb], AF.Sin)

        knT = ppool.tile([128, 2, M], BF16, tag="knT")
        qnT = ppool.tile([128, 2, QH], BF16, tag="qnT")
        lnscale = cpool.tile([128, 1], F32, tag="lnscale")
        nc.vector.memset(lnscale[:], math.log(SCALE))
        zero_c = cpool.tile([128, 1], F32, tag="zero_c")
        nc.vector.memset(zero_c[:], 0.0)
        # scalar-written fences force Sin -> Ln -> Exp engine order so the
        # lower_act pass emits one table load per set (no thrashing).
        # Each fence is Copy(0*x + bias) of the prior phase's last output:
        # zero/const valued, but data-dependent so the scheduler keeps order.
        one_c = cpool.tile([128, 1], F32, tag="one_c")
        nc.vector.memset(one_c[:], 1.0)
        scale_c = cpool.tile([128, 1], F32, tag="scale_c")
        nc.vector.memset(scale_c[:], SCALE)
        fence_ln = cpool.tile([128, 1], F32, tag="fence_ln")
        act(fence_ln[:], cs[1][:, 1, QH - 1 : QH], AF.Sin, scale=0.0,
            bias=zero_c[:])
        fence_exp0 = cpool.tile([128, 1], F32, tag="fence_exp0")
        fence_expS = cpool.tile([128, 1], F32, tag="fence_expS")

        # qk-norm runs in two passes so the scalar engine sees all Ln acts
        # then all Exp acts (one table load each).
        segs = []  # (tb, lnt, out_tile, off, n, ln_bias)

        def norm_pass1_pair(ps_pair, n, idx):
            tb = wpool.tile([128, 2, 512], BF16, tag=f"tb{idx}")
            sq = wpool.tile([128, 2, 512], BF16, tag="sq", bufs=2)
            for dt_i in range(2):
                nc.vector.tensor_copy(tb[:, dt_i, 0:n], ps_pair[dt_i][:, 0:n])
                nc.vector.tensor_mul(
                    sq[:, dt_i, 0:n], tb[:, dt_i, 0:n], tb[:, dt_i, 0:n]
                )
            sq_ps = pso.tile([128, 512], F32, tag="op0")
            for dt_i in range(2):
                mm(
                    sq_ps[0:8, 0:n],
                    bones[:, dt_i, :],
                    sq[:, dt_i, 0:n],
                    start=(dt_i == 0),
                    stop=(dt_i == 1),
                )
            lnt = wpool.tile([8, 512], F32, tag=f"lnt{idx}")
            act(lnt[:, 0:n], sq_ps[0:8, 0:n], AF.Ln, bias=fence_ln[0:8, :])
            return tb, lnt

        def norm_pass1(ps, n, idx):
            tb = wpool.tile([128, 2, 512], BF16, tag=f"tb{idx}")
            nc.vector.tensor_copy(tb[:, :, 0:n], ps[:, :, 0:n])
            sq = wpool.tile([128, 2, 512], BF16, tag="sq", bufs=2)
            nc.vector.tensor_mul(sq[:, :, 0:n], tb[:, :, 0:n], tb[:, :, 0:n])
            sq_ps = pso.tile([128, 512], F32, tag="op0")
            for dt_i in range(2):
                mm(
                    sq_ps[0:8, 0:n],
                    bones[:, dt_i, :],
                    sq[:, dt_i, 0:n],
                    start=(dt_i == 0),
                    stop=(dt_i == 1),
                )
            lnt = wpool.tile([8, 512], F32, tag=f"lnt{idx}")
            act(lnt[:, 0:n], sq_ps[0:8, 0:n], AF.Ln, bias=fence_ln[0:8, :])
            return tb, lnt

        # ---- K projection + norm pass 1 (1-bank psum tiles so phase A
        # keeps the psc pool to itself) ----
        for ci, (ko, kb) in enumerate(K_CHUNKS):
            kps = []
            for dt_i in range(2):
                kp = pso.tile([128, 512], F32, tag=f"op{dt_i}", name=f"kp{dt_i}")
                for kt in range(2):
                    mm(
                        kp[:, 0:kb],
                        wk[:, kt, dt_i * 128 : (dt_i + 1) * 128],
                        tokT[:, kt, ko : ko + kb],
                        start=(kt == 0),
                        stop=(kt == 1),
                    )
                kps.append(kp)
            tb, lnt = norm_pass1_pair(kps, kb, f"k{ci}")
            segs.append((tb, lnt, knT, ko, kb, zero_c))

        # ---- V (token-major) with ones column:  vsb[mc] = (128, 8, 33) ----
        vsb = []
        for mc in range(8):
            ps = pso.tile([128, 512], F32, tag="op1")
            for kt in range(2):
                mm(
                    ps[:, 0:256],
                    tokT[:, kt, mc * 128 : (mc + 1) * 128],
                    wv[:, kt, :],
                    start=(kt == 0),
                    stop=(kt == 1),
                )
            vt = ppool.tile([128, 8, 33], BF16, tag=f"v{mc}")
            nc.vector.memset(vt[:, :, 0:1], 1.0)
            nc.vector.tensor_copy(
                vt[:, :, 1:33], ps[:, 0:256].rearrange("p (h e) -> p h e", h=8)
            )
            vsb.append(vt)


        # ---- phase B: coord proj (bias folded) + RoPE (DVE only) ----
        roped = wpool.tile([128, 2, QH], BF16, tag="roped")
        for qo, qb in Q_BLOCKS:
            pq = psc.tile([128, 2, 512], F32, tag="sc")
            pqs = psc.tile([128, 2, 512], F32, tag="sc")
            for dt_i in range(2):
                for w_t, ps_t in ((wc3, pq), (wcsw3, pqs)):
                    mm(
                        ps_t[:, dt_i, 0:qb],
                        w_t[:, dt_i * 128 : (dt_i + 1) * 128],
                        posAh[:, qo : qo + qb],
                        start=True,
                        stop=False,
                    )
                    mm(
                        ps_t[:, dt_i, 0:qb],
                        w_t[:, dt_i * 128 : (dt_i + 1) * 128],
                        posAl[:, qo : qo + qb],
                        start=False,
                        stop=True,
                    )
            qin = wpool.tile([128, 2, 512], BF16, tag="qin", bufs=2)
            qins = wpool.tile([128, 2, 512], BF16, tag="qins", bufs=2)
            act(qin[:, :, 0:qb], pq[:, :, 0:qb], AF.Copy)
            act(qins[:, :, 0:qb], pqs[:, :, 0:qb], AF.Copy)
            t1 = wpool.tile([128, 2, 512], BF16, tag="t1", bufs=2)
            t2 = wpool.tile([128, 2, 512], BF16, tag="t2", bufs=2)
            for dt_i in range(2):
                # dt0 <- x-axis tables, dt1 <- y-axis
                nc.vector.tensor_mul(
                    t1[:, dt_i, 0:qb],
                    qin[:, dt_i, 0:qb],
                    cs[dt_i][:, 1, qo : qo + qb],
                )
                nc.vector.tensor_mul(
                    t2[:, dt_i, 0:qb],
                    qins[:, dt_i, 0:qb],
                    cs[dt_i][:, 0, qo : qo + qb],
                )
            nc.vector.tensor_add(
                roped[:, :, qo : qo + qb], t1[:, :, 0:qb], t2[:, :, 0:qb]
            )

        # ---- Q projection + norm pass 1 ----
        for qi, (qo, qb) in enumerate(Q_BLOCKS):
            ps = psc.tile([128, 2, 512], F32, tag="sc")
            for dt_i in range(2):
                for kt in range(2):
                    mm(
                        ps[:, dt_i, 0:qb],
                        wq[:, kt, dt_i * 128 : (dt_i + 1) * 128],
                        roped[:, kt, qo : qo + qb],
                        start=(kt == 0),
                        stop=(kt == 1),
                    )
            tb, lnt = norm_pass1(ps, qb, f"q{qi}")
            segs.append((tb, lnt, qnT, qo, qb, lnscale))

        # ---- norm pass 2: all Exp acts, then expand + scale ----
        last_lnt = segs[-1][1]
        # Ln(0*x + 1) == 0 and Ln(0*x + SCALE) == ln(SCALE); both stay in
        # the natural_log table set while depending on the last Ln output.
        act(fence_exp0[0:8, :], last_lnt[0:8, 0:1], AF.Ln, scale=0.0,
            bias=one_c[0:8, :])
        act(fence_expS[0:8, :], last_lnt[0:8, 0:1], AF.Ln, scale=0.0,
            bias=scale_c[0:8, :])
        for si, (tb, lnt, out_t, off, n, ln_bias) in enumerate(segs):
            ln_bias = fence_expS if ln_bias is lnscale else fence_exp0
            invn = wpool.tile([8, 512], BF16, tag="invn", bufs=3)
            act(
                invn[:, 0:n], lnt[:, 0:n], AF.Exp, scale=-0.5, bias=ln_bias[0:8, :]
            )
            for dt_i in range(2):
                psx = pso.tile([128, 512], F32, tag="op1")
                mm(psx[:, 0:n], exp8[:, dt_i, :], invn[:, 0:n])
                nc.vector.tensor_mul(
                    out_t[:, dt_i, off : off + n], tb[:, dt_i, 0:n], psx[:, 0:n]
                )

        # ---- main attention loop ----
        osb = []  # per pair (128, QH) bf16, rows 0/64 = sumexp
        for j in range(4):
            t = ppool.tile([128, QH], BF16, tag=f"osb{j}")
            osb.append(t)

        for qo, qb in Q_BLOCKS:
            for half in range(2):
                dt_i = half
                o_ps = {}
                for jj in range(2):
                    j = 2 * half + jj
                    o_ps[j] = pso.tile(
                        [128, 512], F32, tag=f"op{jj}", name=f"ops{j}"
                    )
                pend = []

                def emit_attend(item):
                    if item[0] == "main":
                        _, j, mc, e_t = item
                        pairs = [(j, e_t[:, 0, 0:qb], e_t[:, 1, 0:qb], mc)]
                    else:
                        _, mc, e_t = item
                        pairs = [
                            (
                                2 * half + jj,
                                e_t[:, jj, 0:qb],
                                e_t[:, jj, 128 : 128 + qb],
                                mc,
                            )
                            for jj in range(2)
                        ]
                    for j, e0, e1, mc in pairs:
                        mm(
                            o_ps[j][0:33, 0:qb],
                            vsb[mc][:, 2 * j, :],
                            e0,
                            start=(mc == 0),
                            stop=(mc == 7),
                            tile_position=(0, 0),
                        )
                        mm(
                            o_ps[j][64:97, 0:qb],
                            vsb[mc][:, 2 * j + 1, :],
                            e1,
                            start=(mc == 0),
                            stop=(mc == 7),
                            tile_position=(0, 64),
                        )

                for mc in range(8):
                    # one cluster: both head-pairs x one m-chunk; each head
                    # gets its own PSUM bank so the 4 row-tiled score mms
                    # can drain concurrently
                    Xs = []
                    for jj in range(2):
                        X = psc.tile(
                            [128, 2, 512], F32, tag="sc", name=f"x{jj}"
                        )
                        Xs.append(X)
                    for jj in range(2):
                        for hh in range(2):
                            mm(
                                Xs[jj][:, hh, 0:qb],
                                ident[:],
                                bias_sb[mc][:, qo : qo + qb],
                                start=True,
                                stop=False,
                            )
                    for jj in range(2):
                        j = 2 * half + jj
                        for hh in range(2):
                            h = 2 * j + hh
                            hp = (h % 4) * 32
                            mm(
                                Xs[jj][:, hh, 0:qb],
                                knT[
                                    hp : hp + 32,
                                    dt_i,
                                    mc * 128 : (mc + 1) * 128,
                                ],
                                qnT[hp : hp + 32, dt_i, qo : qo + qb],
                                start=False,
                                stop=True,
                                tile_position=(hp, 0),
                            )
                    for jj in range(2):
                        j = 2 * half + jj
                        e_t = epool.tile([128, 2, 512], BF16, tag="E")
                        act(e_t[:, :, 0:qb], Xs[jj][:, :, 0:qb], AF.Exp)
                        pend.append(("main", j, mc, e_t))
                    if len(pend) > 2:
                        emit_attend(pend.pop(0))
                        emit_attend(pend.pop(0))
                while pend:
                    emit_attend(pend.pop(0))
                for jj in range(2):
                    j = 2 * half + jj
                    nc.vector.tensor_copy(
                        osb[j][:, qo : qo + qb], o_ps[j][:, 0:qb]
                    )

        # ---- softmax denominators: gather row 0 of each head, invert ----
        sumE = wpool.tile([8, QH], BF16, tag="sumE")
        for h in range(8):
            j, r = h // 2, 64 * (h % 2)
            dma(sumE[h : h + 1, :], osb[h // 2][r : r + 1, :])
        lnZ = wpool.tile([8, QH], F32, tag="lnZ")
        act(lnZ[:], sumE[:], AF.Ln)
        invSb = wpool.tile([8, QH], BF16, tag="invSb")
        act(invSb[:], lnZ[:], AF.Exp, scale=-1.0)

        for qo, qb in Q_BLOCKS:
            for j in range(4):
                ps = pso.tile([128, 512], F32, tag=f"op{j % 2}")
                mm(ps[:, 0:qb], expP[:, j, :], invSb[:, qo : qo + qb])
                nc.vector.tensor_mul(
                    osb[j][:, qo : qo + qb], osb[j][:, qo : qo + qb], ps[:, 0:qb]
                )

        # ---- output projections + residual (all biases via matmul) ----
        for qo, qb in Q_BLOCKS:
            ps = psc.tile([128, 2, 512], F32, tag="sc")
            for dt_i in range(2):
                for j in range(4):
                    mm(
                        ps[:, dt_i, 0:qb],
                        wo_aug[:, j, dt_i * 128 : (dt_i + 1) * 128],
                        osb[j][:, qo : qo + qb],
                        start=(j == 0),
                        stop=(j == 3),
                    )
            o1b = wpool.tile([128, 2, 512], BF16, tag="o1b", bufs=2)
            nc.vector.tensor_copy(o1b[:, :, 0:qb], ps[:, :, 0:qb])
            ps2 = psc.tile([128, 2, 512], F32, tag="sc")
            for dt_i in range(2):
                for kt in range(2):
                    mm(
                        ps2[:, dt_i, 0:qb],
                        wout[:, kt, dt_i * 128 : (dt_i + 1) * 128],
                        o1b[:, kt, 0:qb],
                        start=(kt == 0),
                        stop=False,
                    )
                mm(
                    ps2[:, dt_i, 0:qb],
                    woutb[:, dt_i, :],
                    ones_q[:, 0:qb],
                    start=False,
                    stop=True,
                )
            res = wpool.tile([128, 2, 512], F32, tag="res", bufs=2)
            nc.vector.tensor_add(
                res[:, :, 0:qb], ps2[:, :, 0:qb], fmapT[:, :, qo : qo + qb]
            )
            for dt_i in range(2):
                dma(
                    out_dram[dt_i * 128 : (dt_i + 1) * 128, qo : qo + qb],
                    res[:, dt_i, 0:qb],
                )


def build_in_maps(inputs):
    consts = _host_constants(
        np.asarray(inputs["Wq"], np.float32),
        np.asarray(inputs["Wk"], np.float32),
        np.asarray(inputs["Wv"], np.float32),
        np.asarray(inputs["Wo"], np.float32),
        np.asarray(inputs["W_out_w"], np.float32),
        np.asarray(inputs["W_out_b"], np.float32),
        np.asarray(inputs["W_coord_w"], np.float32),
        np.asarray(inputs["W_coord_b"], np.float32),
    )
    track_tokens = np.asarray(inputs["track_tokens"], np.float32)
    feature_map = np.asarray(inputs["feature_map"], np.float32)
    feature_positions = np.asarray(inputs["feature_positions"], np.float32)
    spatial_bias = np.asarray(inputs["spatial_bias"], np.float32)

    in_maps = []
    for c in range(N_CORES):
        t, half = c // 2, c % 2
        qsl = slice(half * QH, (half + 1) * QH)
        m = dict(consts)
        m["tokT"] = _bf(track_tokens[t].T)
        pos = feature_positions[t, qsl].T  # (2, QH)
        posA = np.concatenate([pos, np.ones((1, QH), np.float32)], axis=0)
        ph = _bf(posA)
        m["posAh"] = ph
        m["posAl"] = _bf(posA - np.asarray(ph, np.float32))
        m["biasT"] = _bf(spatial_bias[t][:, qsl])
        m["fmapT"] = _f32(feature_map[t, qsl].T)
        in_maps.append(m)
    return in_maps


def kernel(
    track_tokens,
    feature_map,
    feature_positions,
    spatial_bias,
    Wq,
    Wk,
    Wv,
    Wo,
    W_out_w,
    W_out_b,
    W_coord_w,
    W_coord_b,
):
    global _NC_CACHE
    in_maps = build_in_maps(
        dict(
            track_tokens=track_tokens,
            feature_map=feature_map,
            feature_positions=feature_positions,
            spatial_bias=spatial_bias,
            Wq=Wq,
            Wk=Wk,
            Wv=Wv,
            Wo=Wo,
            W_out_w=W_out_w,
            W_out_b=W_out_b,
            W_coord_w=W_coord_w,
            W_coord_b=W_coord_b,
        )
    )

    if _NC_CACHE is None:
        _NC_CACHE = _build_nc()
    res = run_bass_kernel_spmd(_NC_CACHE, in_maps, core_ids=list(range(N_CORES)))

    out = np.empty((T, HW, DF), np.float32)
    for c in range(N_CORES):
        t, half = c // 2, c % 2
        qsl = slice(half * QH, (half + 1) * QH)
        out[t, qsl, :] = res.results[c]["out"].T
    return out
